# revision 1
# baseline (speedup 1.0000x reference)
"""Fused Conv3x3-InstanceNorm-ReLU x2 block for Trainium2 (fp16 path).

Data-parallel over 8 NeuronCores (one batch sample per core). Per-core:

  pass A: conv1 as row-pair matmuls (fp16, K=128 = 4 row-shifted Cin
          blocks, M=128 = 2 out rows x 64 Cout, N=320, fp32 PSUM).
          x is host-tiled into 20 contiguous [128, 8*324] fp16 groups
          (pads and edge rows pre-zeroed) so pass A needs exactly 20
          big DMAs. Pairs are processed in couples sharing a 2-bank
          PSUM tile: one strided ACT op evacuates both pairs into a
          big fp16 SBUF arena (amortizing ACT's fixed op cost); DVE
          bn_stats records (count, mean, M2) per pair and partition.
  norm1:  h = relu(y1 - mu1) (DVE, fp16 4x mode); the rsqrt scale s1
          is folded into the conv2 weights per input channel.
  pass B: conv2 on normalized pairs (2 K-blocks of 64 ch, 6 matmuls).
          y2 lands fp16 IN PLACE over the arena slot its eA input just
          retired -- zero spill, zero extra SBUF. Row 0/319 singles run
          after the pair loop so PE enters pass B sooner.
  stats:  per-half bn_aggr (equal-count records per half) + count-
          weighted combine across the two row-parity halves.
  pass C: out = relu((y2 - mu2) * s2), alternating ACT relu / DVE
          2-op form, gathered 8 pairs per [128, 8*320] fp16 tile and
          stored with 2 DMAs per tile (one per row parity). The f16
          wire output is upcast to f32 on the host.

All SBUF pair slots use a 324-element stride with data at offset 2 so
fp16 interiors are 4-byte aligned (enables DVE 2x/4x perf modes).
Weights are pre-transposed on the host (w1t [Cin,3,3,Cout] fp16, w2t
[Cout,3,3,Cout] f32) so weight DMAs have 128-256 B contiguous runs.
Conv biases b1/b2 cancel under InstanceNorm (affine=False) and are
accepted but unused.
"""
import sys
sys.path.insert(0, '/opt/trn_rl_repo')
import contextlib
import numpy as np
import concourse.bacc as bacc_mod
import concourse.tile as tile
import concourse.mybir as mybir
from concourse.ap import AP
from concourse.bass_utils import run_bass_kernel_spmd

f32 = mybir.dt.float32
f32r = mybir.dt.float32r
f16 = mybir.dt.float16
AF = mybir.ActivationFunctionType
OP = mybir.AluOpType

B, CIN, COUT, H, W = 8, 32, 64, 320, 320
WP = W + 2            # matmul rhs window width
WPS = W + 4           # storage stride per pair slot (interior 4B-aligned)
HW = H * W
EPS = 1e-5
NP = H // 2          # 160 conv1 row pairs (h = 0,2,...,318)
NPB = H // 2 - 1     # 159 conv2 row pairs (h = 1,3,...,317)
INV_HW = 1.0 / HW
XG = 8               # conv1 pairs per batched x DMA
COG = 8              # pass-C pairs per batched out DMA
HPOOL = 8            # rotating normalized-h tiles

_CACHE = {}


NG = NP // XG        # 20 batched x-load groups


def _build(repeat=0):
    nc = bacc_mod.Bacc("TRN2", target_bir_lowering=False)
    # host-tiled x: xg[g, j*32+c, s*WP+1+w] = x[c, 2*(8g+s)-1+j, w], 0 padded
    x_d = nc.dram_tensor("xg", [NG, 128, XG * WPS], f16, kind="ExternalInput")
    w1t_d = nc.dram_tensor("w1t", [CIN, 3, 3, COUT], f16, kind="ExternalInput")
    w2t_d = nc.dram_tensor("w2t", [COUT, 3, 3, COUT], f32, kind="ExternalInput")
    out_d = nc.dram_tensor("out", [COUT, H, W], f16, kind="ExternalOutput")

    with tile.TileContext(nc) as tc:
        with contextlib.ExitStack() as ctx:
            wp = ctx.enter_context(tc.tile_pool(name="wp", bufs=1))
            xp = ctx.enter_context(tc.tile_pool(name="xp", bufs=3))
            scr = ctx.enter_context(tc.tile_pool(name="scr", bufs=2))
            cop = ctx.enter_context(tc.tile_pool(name="cop", bufs=4))
            psp = ctx.enter_context(tc.tile_pool(name="psp", bufs=4, space="PSUM"))

            def body(_iv=None):
                zt = wp.tile([128, 128], f32, tag="zt", name="zt", bufs=1)
                nc.gpsimd.memset(zt[:], 0.0)

                bigE = wp.tile([128, NP * WPS], f16, tag="bigE", name="bigE", bufs=1)
                bigE3 = bigE[:].rearrange("p (k w) -> p k w", k=NP)
                # zero every pad column {322k, 322k+321} in one strided memset
                nc.gpsimd.memset(bigE3[:, :, 1:WP + 1:WP - 1], 0.0)

                # ---------------- conv1 weights ---------------------------
                # lhsT[(j,c),(r,o)] = w1[o,c,j-r,kw] = w1t[c, j-r, kw, o]
                # (virtual rows -1/320 are zero in the host-tiled x, so the
                # same "mid" tiles are correct for the edge pairs too)
                lw1 = {}
                for kw in range(3):
                    t = wp.tile([128, 128], f16, tag=f"lw1m{kw}", name=f"lw1m{kw}")
                    nc.vector.tensor_copy(t[:], zt[:])
                    # col half r=0: rows 0:96 <- a=0..2
                    nc.scalar.dma_start(
                        t[0:96, 0:64],
                        AP(w1t_d[:].tensor, kw * 64,
                           [[3 * 64, 3], [9 * 64, 32], [1, 64]]))
                    # col half r=1: rows 32:128 <- a=0..2
                    nc.scalar.dma_start(
                        t[32:128, 64:128],
                        AP(w1t_d[:].tensor, kw * 64,
                           [[3 * 64, 3], [9 * 64, 32], [1, 64]]))
                    lw1[kw] = t

                # ---------------- conv2 weight staging (f32, pre-scale) ---
                # A[(u,i),(r,o)] = w2[o,i,u-r] (u-r in {0,1}) = w2t[i,u-r,kw,o]
                # B[(v,i),(r,o)] = w2[o,i,v+2-r]              = w2t[i,v+2-r,kw,o]
                w2st = {}
                for kw in range(3):
                    sA = wp.tile([128, 128], f32, tag=f"w2sA{kw}", name=f"w2sA{kw}")
                    nc.vector.tensor_copy(sA[0:64, 64:128], zt[0:64, 64:128])
                    # r=0 col half: rows (u)*64, u=0..1 <- kh=u
                    nc.gpsimd.dma_start(
                        sA[:, 0:64],
                        AP(w2t_d[:].tensor, kw * 64,
                           [[3 * 64, 2], [9 * 64, 64], [1, 64]]))
                    # r=1 col half: rows 64:128 (u=1) <- kh=0
                    nc.gpsimd.dma_start(
                        sA[64:128, 64:128],
                        AP(w2t_d[:].tensor, kw * 64, [[9 * 64, 64], [1, 64]]))
                    w2st[("A", kw)] = sA
                    sB = wp.tile([128, 128], f32, tag=f"w2sB{kw}", name=f"w2sB{kw}")
                    nc.vector.tensor_copy(sB[64:128, 0:64], zt[64:128, 0:64])
                    # r=0 col half: rows 0:64 (v=0) <- kh=2
                    nc.gpsimd.dma_start(
                        sB[0:64, 0:64],
                        AP(w2t_d[:].tensor, 2 * 3 * 64 + kw * 64,
                           [[9 * 64, 64], [1, 64]]))
                    # r=1 col half: rows (v)*64, v=0..1 <- kh=v+1
                    nc.gpsimd.dma_start(
                        sB[:, 64:128],
                        AP(w2t_d[:].tensor, 3 * 64 + kw * 64,
                           [[3 * 64, 2], [9 * 64, 64], [1, 64]]))
                    w2st[("B", kw)] = sB
                    # S0: row 0 single (taps kh=1,2 from rows 0,1)
                    s0 = wp.tile([128, 64], f32, tag=f"w2s0{kw}", name=f"w2s0{kw}")
                    nc.gpsimd.dma_start(
                        s0[:, :],
                        AP(w2t_d[:].tensor, 3 * 64 + kw * 64,
                           [[3 * 64, 2], [9 * 64, 64], [1, 64]]))
                    w2st[("S0", kw)] = s0
                    # S9: row H-1 single (taps kh=0,1 from rows 318,319)
                    s9 = wp.tile([128, 64], f32, tag=f"w2s9{kw}", name=f"w2s9{kw}")
                    nc.gpsimd.dma_start(
                        s9[:, :],
                        AP(w2t_d[:].tensor, kw * 64,
                           [[3 * 64, 2], [9 * 64, 64], [1, 64]]))
                    w2st[("S9", kw)] = s9

                # bn_stats records: 6 f32 per (pair, partition)
                st1 = wp.tile([128, NP * 6], f32, tag="st1", name="st1")
                st2 = wp.tile([128, (NPB + 2) * 6], f32, tag="st2", name="st2")

                # ---------------- pass A: conv1 + stats --------------------
                # couples: 2 pairs share a [128,1024] 2-bank psum tile and
                # one strided ACT evac (amortizes ACT's fixed op cost)
                def passA_couple(k0, rhs):
                    ps2 = psp.tile([128, 1024], f32, tag="pp", name=f"psA{k0}")
                    for h2 in range(2):
                        off = (k0 % XG + h2) * WPS + 1
                        for kw in range(3):
                            nc.tensor.matmul(ps2[:, 512 * h2:512 * h2 + W],
                                             lw1[kw][:],
                                             rhs[:, off + kw:off + kw + W],
                                             start=(kw == 0), stop=(kw == 2))
                    ev = ps2[:].rearrange("p (b w) -> p b w", b=2)[:, :, 0:W]
                    nc.scalar.activation(bigE3[:, k0:k0 + 2, 2:W + 2], ev,
                                         AF.Copy)
                    for k in (k0, k0 + 1):
                        nc.vector.bn_stats(st1[:, 6 * k:6 * k + 6],
                                           bigE3[:, k, 2:W + 2])

                for gi in range(NG):
                    xg = xp.tile([128, XG * WPS], f16, tag="xg", name=f"xg{gi}")
                    nc.sync.dma_start(xg[:], x_d[gi])
                    for s2 in range(0, XG, 2):
                        passA_couple(XG * gi + s2, xg)

                # ---------------- stats -> mu, 1/sd ------------------------
                def stats(st, n0, n1, pfx):
                    # st: [128, ncols*6] bn_stats records; partitions 0:64
                    # hold n0 records, 64:128 hold n1 (all records n=320,
                    # bn_aggr's variance combine requires equal counts)
                    agg = wp.tile([128, 2], f32, tag=f"{pfx}agg", name=f"{pfx}agg")
                    nc.vector.bn_aggr(agg[0:64, :], st[0:64, 0:6 * n0])
                    nc.vector.bn_aggr(agg[64:128, :], st[64:128, 0:6 * n1])
                    mean = agg[:, 0:1]
                    var = agg[:, 1:2]
                    ex2 = wp.tile([128, 1], f32, tag=f"{pfx}ex2", name=f"{pfx}ex2")
                    nc.vector.tensor_tensor(ex2[:], mean, mean, OP.mult)
                    nc.vector.tensor_tensor(ex2[:], var, ex2[:], OP.add)
                    fa = wp.tile([64, 1], f32, tag=f"{pfx}fa", name=f"{pfx}fa")
                    fb = wp.tile([64, 1], f32, tag=f"{pfx}fb", name=f"{pfx}fb")
                    nc.sync.dma_start(fa[:], agg[64:128, 0:1])
                    nc.sync.dma_start(fb[:], ex2[64:128, :])
                    w0, w1 = n0 / (n0 + n1), n1 / (n0 + n1)
                    mu = wp.tile([64, 1], f32, tag=f"{pfx}mu", name=f"{pfx}mu")
                    e2 = wp.tile([64, 1], f32, tag=f"{pfx}e2", name=f"{pfx}e2")
                    t = wp.tile([64, 1], f32, tag=f"{pfx}t", name=f"{pfx}t")
                    nc.vector.tensor_scalar(mu[:], mean[0:64], w0, None, OP.mult)
                    nc.vector.tensor_scalar(t[:], fa[:], w1, None, OP.mult)
                    nc.vector.tensor_tensor(mu[:], mu[:], t[:], OP.add)
                    nc.vector.tensor_scalar(e2[:], ex2[0:64, :], w0, None, OP.mult)
                    nc.vector.tensor_scalar(t[:], fb[:], w1, None, OP.mult)
                    nc.vector.tensor_tensor(e2[:], e2[:], t[:], OP.add)
                    varo = wp.tile([64, 1], f32, tag=f"{pfx}varo", name=f"{pfx}varo")
                    nc.vector.tensor_tensor(varo[:], mu[:], mu[:], OP.mult)
                    nc.vector.tensor_tensor(varo[:], e2[:], varo[:], OP.subtract)
                    nc.vector.tensor_scalar(varo[:], varo[:], EPS, None, OP.add)
                    sd = wp.tile([64, 1], f32, tag=f"{pfx}sd", name=f"{pfx}sd")
                    nc.scalar.activation(sd[:], varo[:], AF.Sqrt)
                    s = wp.tile([64, 1], f32, tag=f"{pfx}s", name=f"{pfx}s")
                    nc.vector.reciprocal(s[:], sd[:])
                    return mu, s

                def bcast128(src64, tag):
                    t = wp.tile([128, 1], f32, tag=tag, name=tag)
                    nc.sync.dma_start(t[0:64, :], src64[:])
                    nc.sync.dma_start(t[64:128, :], src64[:])
                    return t

                mu1, s1 = stats(st1, NP, NP, "st1")
                negmu1 = wp.tile([64, 1], f32, tag="negmu1", name="negmu1")
                nc.vector.tensor_scalar(negmu1[:], mu1[:], -1.0, None, OP.mult)
                nmu1v = bcast128(negmu1, "nmu1v")
                s1v = bcast128(s1, "s1v")

                # scale staged conv2 weights by s1 (per input channel = partition)
                lw2 = {}
                for key, st in w2st.items():
                    cols = st.shape[-1]
                    t = wp.tile([128, cols], f16, tag=f"lw2{key[0]}{key[1]}",
                                name=f"lw2{key[0]}{key[1]}")
                    nc.vector.tensor_scalar(t[:], st[:], s1v[:, 0:1], None, OP.mult)
                    lw2[key] = t

                # ---------------- pass B: conv2 + stats --------------------
                # slot 0 is pinned to h[0] (the row-0 single consumes it after
                # the pair loop); slots 1..HPOOL rotate for i >= 1
                bigH = wp.tile([128, (HPOOL + 1) * WPS], f16, tag="bigH",
                               name="bigH", bufs=1)
                bigH3 = bigH[:].rearrange("p (k w) -> p k w", k=HPOOL + 1)
                nc.gpsimd.memset(bigH3[:, :, 1:WP + 1:WP - 1], 0.0)
                hk = {}

                def norm(i):
                    # h[i] = relu(e[i] - mu1), fp16, rotating slot
                    sl = 0 if i == 0 else 1 + (i - 1) % HPOOL
                    nc.vector.tensor_scalar(bigH3[:, sl, 2:W + 2],
                                            bigE3[:, i, 2:W + 2],
                                            nmu1v[:, 0:1], 0.0, OP.add, OP.max)
                    hk[i] = bigH[:, sl * WPS + 1:sl * WPS + 1 + WP]

                norm(0)
                norm(1)
                norm(2)
                h0 = hk[0]   # slot 0 is pinned; row-0 single runs post-loop

                for kb in range(NPB):
                    if kb + 3 <= NP - 1:
                        norm(kb + 3)
                    eA, eB = hk.pop(kb), hk[kb + 1]
                    ps = psp.tile([128, 1024], f32, tag="pp",
                                  name=f"psB{kb}")[:, 0:W]
                    for kw in range(3):
                        nc.tensor.matmul(ps[:, :], lw2[("A", kw)][:],
                                         eA[:, kw:kw + W],
                                         start=(kw == 0), stop=False)
                    for kw in range(3):
                        nc.tensor.matmul(ps[:, :], lw2[("B", kw)][:],
                                         eB[:, kw:kw + W],
                                         start=False, stop=(kw == 2))
                    # y2 pair kb lands bf16 over the e slot it just retired
                    nc.scalar.activation(bigE3[:, kb, 2:W + 2], ps[:], AF.Copy)
                    nc.vector.bn_stats(st2[:, 6 * kb:6 * kb + 6],
                                       bigE3[:, kb, 2:W + 2])

                # single row 0: taps kh=1,2 from rows 0,1 (h[0])
                ps0 = psp.tile([128, 1024], f32, tag="pp",
                               name="psS0")[0:64, 0:W]
                for kw in range(3):
                    nc.tensor.matmul(ps0[:, :], lw2[("S0", kw)][:],
                                     h0[:, kw:kw + W],
                                     start=(kw == 0), stop=(kw == 2))
                y0 = wp.tile([64, W], f32, tag="ys0", name="ys0")
                nc.scalar.activation(y0[:], ps0[:], AF.Copy)
                nc.vector.bn_stats(st2[0:64, 6 * NPB:6 * NPB + 6], y0[:])

                # single row 319: taps kh=0,1 from rows 318,319 (h[159])
                e9 = hk[NP - 1]
                ps9 = psp.tile([128, 1024], f32, tag="pp",
                               name="psS9")[0:64, 0:W]
                for kw in range(3):
                    nc.tensor.matmul(ps9[:, :], lw2[("S9", kw)][:],
                                     e9[:, kw:kw + W],
                                     start=(kw == 0), stop=(kw == 2))
                y9 = wp.tile([64, W], f32, tag="ys9", name="ys9")
                nc.scalar.activation(y9[:], ps9[:], AF.Copy)
                nc.vector.bn_stats(st2[0:64, 6 * (NPB + 1):6 * (NPB + 2)], y9[:])

                # ---------------- stats2 -> s2, t2 = -mu2*s2 ---------------
                mu2, s2 = stats(st2, NPB + 2, NPB, "st2")
                t2 = wp.tile([64, 1], f32, tag="t2", name="t2")
                nc.vector.tensor_tensor(t2[:], mu2[:], s2[:], OP.mult)
                nc.vector.tensor_scalar(t2[:], t2[:], -1.0, None, OP.mult)
                s2v = bcast128(s2, "s2v")
                t2v = bcast128(t2, "t2v")
                negmu2 = wp.tile([64, 1], f32, tag="negmu2", name="negmu2")
                nc.vector.tensor_scalar(negmu2[:], mu2[:], -1.0, None, OP.mult)
                nmu2v = bcast128(negmu2, "nmu2v")

                # ---------------- pass C: out = relu(y2*s2 + t2) -----------
                co0 = wp.tile([64, W], f16, tag="co0", name="co0")
                nc.scalar.activation(co0[:], y0[:], AF.Relu,
                                     bias=t2v[0:64, 0:1], scale=s2v[0:64, 0:1])
                nc.sync.dma_start(out_d[:, 0, :], co0[:])

                cgroups = []
                kb0 = 0
                while kb0 < NPB:
                    g = min(COG, NPB - kb0)
                    cgroups.append((kb0, g))
                    kb0 += g
                for ci, (kb0, g) in enumerate(cgroups):
                    co = cop.tile([128, COG * W], f16, tag="co", name=f"co{ci}")
                    for q in range(g):
                        kb = kb0 + q
                        dst = co[:, q * W:(q + 1) * W]
                        ysrc = bigE3[:, kb, 2:W + 2]
                        if kb % 5 >= 3:
                            nc.scalar.activation(dst, ysrc, AF.Relu,
                                                 bias=t2v[:, 0:1],
                                                 scale=s2v[:, 0:1])
                        else:
                            # relu((y-mu2)*s2) = max((y + -mu2)*s2, 0), s2>0
                            nc.vector.tensor_scalar(dst, ysrc, nmu2v[:, 0:1],
                                                    None, OP.add)
                            nc.vector.tensor_scalar(dst, dst, s2v[:, 0:1], 0.0,
                                                    OP.mult, OP.max)
                    # dst[c,(q,w)] = out[c, 2*(kb0+q)+1+r, w], one DMA per r
                    co3 = co[:].rearrange("p (q w) -> p q w", w=W)
                    for r in range(2):
                        eng = nc.sync if r == 0 else nc.gpsimd
                        eng.dma_start(
                            AP(out_d[:].tensor, (2 * kb0 + 1 + r) * W,
                               [[HW, COUT], [2 * W, g], [1, W]]),
                            co3[r * 64:(r + 1) * 64, 0:g, :])

                co9 = wp.tile([64, W], f16, tag="co9", name="co9")
                nc.scalar.activation(co9[:], y9[:], AF.Relu,
                                     bias=t2v[0:64, 0:1], scale=s2v[0:64, 0:1])
                nc.sync.dma_start(out_d[:, H - 1, :], co9[:])

            if repeat:
                with tc.For_i(0, repeat, 1, hint_engines=(mybir.EngineType.PE,)):
                    body()
            else:
                body()

    nc.finalize()
    return nc


def _get_nc(repeat=0):
    key = ("nc", repeat)
    if key not in _CACHE:
        _CACHE[key] = _build(repeat)
    return _CACHE[key]


def _tile_x(xi):
    # xg[g, j*32+c, s*WPS+2+w] = x[c, 2*(8g+s)-1+j, w], zero padded, fp16
    # (data starts at slot offset 2 so fp16 interiors are 4B-aligned)
    xpad = np.zeros((CIN, H + 2, W), np.float16)
    xpad[:, 1:H + 1] = xi
    rows = 2 * np.arange(NP)[:, None] + np.arange(4)[None, :]   # [NP,4]
    xt = np.zeros((NP, 4, CIN, WPS), np.float16)
    xt[..., 2:W + 2] = xpad[:, rows, :].transpose(1, 2, 0, 3)
    return np.ascontiguousarray(
        xt.reshape(NG, XG, 128, WPS).transpose(0, 2, 1, 3).reshape(NG, 128, XG * WPS))


def _in_map(xi, w1, w2):
    w1t = np.ascontiguousarray(w1.transpose(1, 2, 3, 0).astype(np.float16))
    w2t = np.ascontiguousarray(w2.transpose(1, 2, 3, 0))
    return {"xg": _tile_x(np.asarray(xi, np.float16)), "w1t": w1t, "w2t": w2t}


def kernel(x, w1, b1=None, w2=None, b2=None, **kw):
    x = np.ascontiguousarray(np.asarray(x, dtype=np.float32))
    w1 = np.ascontiguousarray(np.asarray(w1, dtype=np.float32))
    w2 = np.ascontiguousarray(np.asarray(w2, dtype=np.float32))
    nc = _get_nc()
    in_maps = [_in_map(x[i], w1, w2) for i in range(B)]
    res = run_bass_kernel_spmd(nc, in_maps, list(range(B)), trace=False)
    return np.stack([res.results[i]["out"].astype(np.float32) for i in range(B)],
                    axis=0)



# revision 2
# speedup vs baseline: 1.0845x; 1.0845x over previous
"""Fused Conv3x3-InstanceNorm-ReLU x2 block for Trainium2 (fp16, v2).

Data-parallel over 8 NeuronCores (one batch sample per core). Per-core
restructuring vs v1:

  - weights host-packed (zeros baked in) into 3 flat f16 arenas; 1 DMA
    each, no on-device staging copies. conv2 weights are scaled by the
    per-input-channel rsqrt s1 with ONE tensor_scalar over the arena.
  - stats: bn_stats batched (pass A: per 8-pair x-group; pass B: per 3
    pairs), one bn_aggr per pass; cross-partition (row-parity) combine
    and 64->128 broadcast are done with tiny PE matmuls against
    host-built combine/duplicate matrices instead of DMA round-trips.
  - pass B evacuates TWO pairs per ACT op (couples share a 2-bank PSUM
    tile like pass A), halving ACT op count in the conv2 stream.
  - pass C (out = relu((y2-mu2)*s2)) is split across ACT (1-op
    relu-bias-scale), DVE (2-op tensor_scalar), and Pool couples to
    maximize overlap with the 13 MB output store stream.
"""
import sys
sys.path.insert(0, '/opt/trn_rl_repo')
import contextlib
import numpy as np
import concourse.bacc as bacc_mod
import concourse.tile as tile
import concourse.mybir as mybir
from concourse.ap import AP
from concourse.bass_utils import run_bass_kernel_spmd

f32 = mybir.dt.float32
f16 = mybir.dt.float16
AF = mybir.ActivationFunctionType
OP = mybir.AluOpType

B, CIN, COUT, H, W = 8, 32, 64, 320, 320
WP = W + 2            # matmul rhs window width
WPS = W + 4           # storage stride per pair slot (interior 4B-aligned)
HW = H * W
EPS = 1e-5
NP = H // 2           # 160 conv1 row pairs
NPB = H // 2 - 1      # 159 conv2 row pairs
XG = 8                # conv1 pairs per batched x DMA
NG = NP // XG         # 20 batched x-load groups
COG = 8               # pass-C pairs per batched out DMA
HPOOL = 8             # rotating normalized-h slots (1..8; slot 0 pinned h0)

_CACHE = {}


def _build(repeat=0):
    nc = bacc_mod.Bacc("TRN2", target_bir_lowering=False)
    x_d = nc.dram_tensor("xg", [NG, 128, XG * WPS], f16, kind="ExternalInput")
    # lw1d[kw] = [128, 128] conv1 lhsT tiles (zeros baked)
    lw1_d = nc.dram_tensor("lw1", [128, 3 * 128], f16, kind="ExternalInput")
    # w2s: A0,A1,A2,B0,B1,B2 (128 cols each), S0_0..2, S9_0..2 (64 cols each)
    w2s_d = nc.dram_tensor("w2s", [128, 1152], f16, kind="ExternalInput")
    # aux: CMB1[0:64] CMB2[64:128] CMB2s[128:192] DUP[192:320]
    aux_d = nc.dram_tensor("aux", [128, 320], f16, kind="ExternalInput")
    out_d = nc.dram_tensor("out", [COUT, H, W], f16, kind="ExternalOutput")

    with tile.TileContext(nc) as tc:
        with contextlib.ExitStack() as ctx:
            wp = ctx.enter_context(tc.tile_pool(name="wp", bufs=1))
            xp = ctx.enter_context(tc.tile_pool(name="xp", bufs=3))
            cop = ctx.enter_context(tc.tile_pool(name="cop", bufs=4))
            psp = ctx.enter_context(tc.tile_pool(name="psp", bufs=3, space="PSUM"))
            psc = ctx.enter_context(tc.tile_pool(name="psc", bufs=2, space="PSUM"))

            def body(_iv=None):
                # -------- load inputs (x group 0 first for fast start) -----
                xgs = [xp.tile([128, XG * WPS], f16, tag="xg", name=f"xg{g}")
                       for g in range(NG)]
                lw1 = wp.tile([128, 3 * 128], f16, tag="lw1", name="lw1")
                nc.scalar.dma_start(lw1[:], lw1_d[:])
                nc.sync.dma_start(xgs[0][:], x_d[0])
                w2s = wp.tile([128, 1152], f16, tag="w2s", name="w2s")
                aux = wp.tile([128, 320], f16, tag="aux", name="aux")

                bigE = wp.tile([128, NP * WPS], f16, tag="bigE", name="bigE",
                               bufs=1)
                bigE3 = bigE[:].rearrange("p (k w) -> p k w", k=NP)
                bigH = wp.tile([128, (HPOOL + 1) * WPS], f16, tag="bigH",
                               name="bigH", bufs=1)
                bigH3 = bigH[:].rearrange("p (k w) -> p k w", k=HPOOL + 1)
                nc.gpsimd.memset(bigH3[:, :, 1:WP + 1:WP - 1], 0.0)

                st1 = wp.tile([128, NP * 6], f32, tag="st1", name="st1")
                st2 = wp.tile([128, NPB * 6], f32, tag="st2", name="st2")

                # -------- pass A: conv1 + batched stats --------------------
                def passA_couple(k0, rhs):
                    ps2 = psp.tile([128, 1024], f32, tag="pp", name=f"psA{k0}")
                    for h2 in range(2):
                        off = (k0 % XG + h2) * WPS + 2
                        for kw in range(3):
                            nc.tensor.matmul(
                                ps2[:, 512 * h2:512 * h2 + W],
                                lw1[:, 128 * kw:128 * kw + 128],
                                rhs[:, off + kw - 1:off + kw - 1 + W],
                                start=(kw == 0), stop=(kw == 2))
                    ev = ps2[:].rearrange("p (b w) -> p b w", b=2)[:, :, 0:W]
                    nc.scalar.activation(bigE3[:, k0:k0 + 2, 2:W + 2], ev,
                                         AF.Copy)
                    for k in (k0, k0 + 1):
                        nc.vector.bn_stats(st1[:, 6 * k:6 * k + 6],
                                           bigE3[:, k, 2:W + 2])

                for gi in range(NG):
                    if gi + 1 < NG:
                        nc.sync.dma_start(xgs[gi + 1][:], x_d[gi + 1])
                    if gi == 1:
                        nc.gpsimd.dma_start(w2s[:], w2s_d[:])
                        nc.gpsimd.dma_start(aux[:], aux_d[:])
                    for s2 in range(0, XG, 2):
                        passA_couple(XG * gi + s2, xgs[gi])

                # -------- stats1 -> -mu1, s1 broadcast; scale conv2 w ------
                CMB1 = aux[:, 0:64]
                CMB2 = aux[:, 64:128]
                CMB2s = aux[0:64, 128:192]
                DUP = aux[0:64, 192:320]

                def half_stats(st, ncols, pfx, extra=None):
                    # st: [128, 6*ncols] equal-count records ->
                    # X f16 [128, 2] = (mean, E[y^2]) per partition
                    agg = wp.tile([128, 2], f32, tag=f"{pfx}agg", name=f"{pfx}agg")
                    nc.vector.bn_aggr(agg[:], st[:, 0:6 * ncols])
                    m2 = wp.tile([128, 1], f32, tag=f"{pfx}m2", name=f"{pfx}m2")
                    nc.vector.tensor_tensor(m2[:], agg[:, 0:1], agg[:, 0:1],
                                            OP.mult)
                    nc.vector.tensor_tensor(agg[:, 1:2], agg[:, 1:2], m2[:],
                                            OP.add)
                    X = wp.tile([128, 2], f16, tag=f"{pfx}X", name=f"{pfx}X")
                    nc.vector.tensor_scalar(X[:], agg[:], 1.0, None, OP.mult)
                    return X

                def finish_stats(ps_mq, pfx, want_t2=False):
                    # ps_mq: PSUM [64, 2] = (mu_tot, E[y^2]_tot); returns
                    # broadcast [128, k] f32 (nmu, s[, t2=-mu*s])
                    mq = wp.tile([64, 2], f32, tag=f"{pfx}mq", name=f"{pfx}mq")
                    nc.scalar.activation(mq[:], ps_mq, AF.Copy)
                    mu = mq[:, 0:1]
                    t = wp.tile([64, 1], f32, tag=f"{pfx}t", name=f"{pfx}t")
                    nc.vector.tensor_tensor(t[:], mu, mu, OP.mult)
                    varo = wp.tile([64, 1], f32, tag=f"{pfx}v", name=f"{pfx}v")
                    nc.vector.tensor_tensor(varo[:], mq[:, 1:2], t[:],
                                            OP.subtract)
                    nc.vector.tensor_scalar(varo[:], varo[:], EPS, None, OP.add)
                    sd = wp.tile([64, 1], f32, tag=f"{pfx}sd", name=f"{pfx}sd")
                    nc.scalar.activation(sd[:], varo[:], AF.Sqrt)
                    s = wp.tile([64, 1], f32, tag=f"{pfx}s", name=f"{pfx}s")
                    nc.vector.reciprocal(s[:], sd[:])
                    k = 3 if want_t2 else 2
                    P = wp.tile([64, 3], f16, tag=f"{pfx}P", name=f"{pfx}P")
                    nc.vector.tensor_scalar(P[:, 0:1], mu, -1.0, None, OP.mult)
                    nc.vector.tensor_scalar(P[:, 1:2], s[:], 1.0, None, OP.mult)
                    if want_t2:
                        t2 = wp.tile([64, 1], f32, tag=f"{pfx}t2", name=f"{pfx}t2")
                        nc.vector.tensor_tensor(t2[:], mu, s[:], OP.mult)
                        nc.vector.tensor_scalar(P[:, 2:3], t2[:], -1.0, None,
                                                OP.mult)
                    psb = psc.tile([128, 512], f32, tag="pc", name=f"{pfx}psb")
                    nc.tensor.matmul(psb[:, 0:k], DUP, P[:, 0:k],
                                     start=True, stop=True)
                    nb = wp.tile([128, 3], f32, tag=f"{pfx}nb", name=f"{pfx}nb")
                    nc.scalar.activation(nb[:, 0:k], psb[:, 0:k], AF.Copy)
                    return nb

                X1 = half_stats(st1, NP, "s1")
                ps1 = psc.tile([128, 512], f32, tag="pc", name="ps1cmb")
                nc.tensor.matmul(ps1[0:64, 0:2], CMB1, X1[:], start=True,
                                 stop=True)
                nb1 = finish_stats(ps1[0:64, 0:2], "s1")
                nmu1v, s1v = nb1[:, 0:1], nb1[:, 1:2]

                lw2 = wp.tile([128, 1152], f16, tag="lw2", name="lw2")
                nc.vector.tensor_scalar(lw2[:], w2s[:], s1v, None, OP.mult)
                lwA = {kw: lw2[:, 128 * kw:128 * kw + 128] for kw in range(3)}
                lwB = {kw: lw2[:, 384 + 128 * kw:384 + 128 * kw + 128]
                       for kw in range(3)}
                lwS0 = {kw: lw2[:, 768 + 64 * kw:768 + 64 * kw + 64]
                        for kw in range(3)}
                lwS9 = {kw: lw2[:, 960 + 64 * kw:960 + 64 * kw + 64]
                        for kw in range(3)}

                # -------- pass B: conv2 (couples) + batched stats ----------
                hk = {}

                def norm2(i):
                    # normalize pairs i, i+1 (adjacent rotating slots)
                    sl = 1 + (i - 1) % HPOOL
                    nc.vector.tensor_scalar(
                        bigH3[:, sl:sl + 2, 2:W + 2],
                        bigE3[:, i:i + 2, 2:W + 2],
                        nmu1v, 0.0, OP.add, OP.max)
                    hk[i] = bigH[:, sl * WPS + 1:sl * WPS + 1 + WP]
                    hk[i + 1] = bigH[:, (sl + 1) * WPS + 1:(sl + 1) * WPS + 1 + WP]

                def norm1(i):
                    sl = 0 if i == 0 else 1 + (i - 1) % HPOOL
                    nc.vector.tensor_scalar(
                        bigH3[:, sl, 2:W + 2], bigE3[:, i, 2:W + 2],
                        nmu1v, 0.0, OP.add, OP.max)
                    hk[i] = bigH[:, sl * WPS + 1:sl * WPS + 1 + WP]

                norm1(0)
                norm2(1)
                norm2(3)
                h0 = hk[0]

                def conv2_pair(ps, eA, eB):
                    for kw in range(3):
                        nc.tensor.matmul(ps, lwA[kw], eA[:, kw:kw + W],
                                         start=(kw == 0), stop=False)
                    for kw in range(3):
                        nc.tensor.matmul(ps, lwB[kw], eB[:, kw:kw + W],
                                         start=False, stop=(kw == 2))

                for c in range(80):
                    kb = 2 * c
                    ni = kb + 5
                    if ni <= NP - 2:
                        norm2(ni)
                    elif ni == NP - 1:
                        norm1(ni)
                    ps2 = psp.tile([128, 1024], f32, tag="pp", name=f"psB{kb}")
                    npair = 2 if kb + 1 < NPB else 1
                    for h2 in range(npair):
                        conv2_pair(ps2[:, 512 * h2:512 * h2 + W],
                                   hk.pop(kb + h2), hk[kb + h2 + 1])
                    if npair == 2:
                        ev = ps2[:].rearrange("p (b w) -> p b w", b=2)[:, :, 0:W]
                        nc.scalar.activation(bigE3[:, kb:kb + 2, 2:W + 2], ev,
                                             AF.Copy)
                    else:
                        nc.scalar.activation(bigE3[:, kb, 2:W + 2],
                                             ps2[:, 0:W], AF.Copy)
                    for h2 in range(npair):
                        nc.vector.bn_stats(st2[:, 6 * (kb + h2):6 * (kb + h2) + 6],
                                           bigE3[:, kb + h2, 2:W + 2])

                # singles: out rows 0 and H-1 (64 partitions), shared tile
                y09 = wp.tile([64, 2 * W], f32, tag="y09", name="y09")
                ps0 = psp.tile([128, 1024], f32, tag="pp", name="psS09")
                for kw in range(3):
                    nc.tensor.matmul(ps0[0:64, 0:W], lwS0[kw], h0[:, kw:kw + W],
                                     start=(kw == 0), stop=(kw == 2))
                e9 = hk[NP - 1]
                for kw in range(3):
                    nc.tensor.matmul(ps0[0:64, 512:512 + W], lwS9[kw],
                                     e9[:, kw:kw + W],
                                     start=(kw == 0), stop=(kw == 2))
                ev09 = ps0[0:64].rearrange("p (b w) -> p b w", b=2)[:, :, 0:W]
                nc.scalar.activation(
                    y09[:].rearrange("p (b w) -> p b w", b=2), ev09, AF.Copy)
                sts = wp.tile([64, 12], f32, tag="sts", name="sts")
                nc.vector.bn_stats(sts[:, 0:6], y09[:, 0:W])
                nc.vector.bn_stats(sts[:, 6:12], y09[:, W:2 * W])

                # -------- stats2 -> broadcast (-mu2, s2, t2) ---------------
                X2 = half_stats(st2, NPB, "s2")
                aggs = wp.tile([64, 2], f32, tag="aggs", name="aggs")
                nc.vector.bn_aggr(aggs[:], sts[:])
                m2s = wp.tile([64, 1], f32, tag="m2s", name="m2s")
                nc.vector.tensor_tensor(m2s[:], aggs[:, 0:1], aggs[:, 0:1],
                                        OP.mult)
                nc.vector.tensor_tensor(aggs[:, 1:2], aggs[:, 1:2], m2s[:],
                                        OP.add)
                Xs = wp.tile([64, 2], f16, tag="Xs", name="Xs")
                nc.vector.tensor_scalar(Xs[:], aggs[:], 1.0, None, OP.mult)
                ps2c = psc.tile([128, 512], f32, tag="pc", name="ps2cmb")
                nc.tensor.matmul(ps2c[0:64, 0:2], CMB2, X2[:], start=True,
                                 stop=False)
                nc.tensor.matmul(ps2c[0:64, 0:2], CMB2s, Xs[:], start=False,
                                 stop=True)
                nb2 = finish_stats(ps2c[0:64, 0:2], "s2", want_t2=True)
                nmu2v, s2v, t2v = nb2[:, 0:1], nb2[:, 1:2], nb2[:, 2:3]

                # -------- pass C: out = relu((y2-mu2)*s2) + stores ---------
                co0 = wp.tile([64, W], f16, tag="co0", name="co0")
                nc.scalar.activation(co0[:], y09[:, 0:W], AF.Relu,
                                     bias=t2v[0:64], scale=s2v[0:64])
                nc.sync.dma_start(out_d[:, 0, :], co0[:])

                def outC_couple(dst, kb, n, sel):
                    if n > 1:
                        d3 = dst.rearrange("p (q w) -> p q w", w=W)
                        s3 = bigE3[:, kb:kb + n, 2:W + 2]
                    else:
                        d3 = dst
                        s3 = bigE3[:, kb, 2:W + 2]
                    if sel == 0:
                        nc.scalar.activation(d3, s3, AF.Relu, bias=t2v,
                                             scale=s2v)
                    elif sel == 1:
                        nc.vector.tensor_scalar(d3, s3, nmu2v, None, OP.add)
                        nc.vector.tensor_scalar(d3, d3, s2v, 0.0, OP.mult,
                                                OP.max)
                    else:
                        nc.gpsimd.tensor_scalar(d3, s3, nmu2v, None, OP.add)
                        nc.gpsimd.tensor_scalar(d3, d3, s2v, 0.0, OP.mult,
                                                OP.max)

                ci = 0
                kb0 = 0
                while kb0 < NPB:
                    g = min(COG, NPB - kb0)
                    co = cop.tile([128, COG * W], f16, tag="co", name=f"co{ci}")
                    q = 0
                    while q < g:
                        n = min(2, g - q)
                        sel = (0, 1, 1, 0, 1)[(ci * 4 + q // 2) % 5]
                        outC_couple(co[:, q * W:(q + n) * W], kb0 + q, n, sel)
                        q += n
                    co3 = co[:].rearrange("p (q w) -> p q w", w=W)
                    for r in range(2):
                        eng = nc.sync
                        eng.dma_start(
                            AP(out_d[:].tensor, (2 * kb0 + 1 + r) * W,
                               [[HW, COUT], [2 * W, g], [1, W]]),
                            co3[r * 64:(r + 1) * 64, 0:g, :])
                    kb0 += g
                    ci += 1

                co9 = wp.tile([64, W], f16, tag="co9", name="co9")
                nc.scalar.activation(co9[:], y09[:, W:2 * W], AF.Relu,
                                     bias=t2v[0:64], scale=s2v[0:64])
                nc.sync.dma_start(out_d[:, H - 1, :], co9[:])

            if repeat:
                with tc.For_i(0, repeat, 1, hint_engines=(mybir.EngineType.PE,)):
                    body()
            else:
                body()

    nc.finalize()
    return nc


def _get_nc(repeat=0):
    key = ("nc", repeat)
    if key not in _CACHE:
        _CACHE[key] = _build(repeat)
    return _CACHE[key]


def _tile_x(xi):
    # xg[g, j*32+c, s*WPS+2+w] = x[c, 2*(8g+s)-1+j, w], zero padded, fp16
    xpad = np.zeros((CIN, H + 2, W), np.float16)
    xpad[:, 1:H + 1] = xi
    rows = 2 * np.arange(NP)[:, None] + np.arange(4)[None, :]
    xt = np.zeros((NP, 4, CIN, WPS), np.float16)
    xt[..., 2:W + 2] = xpad[:, rows, :].transpose(1, 2, 0, 3)
    return np.ascontiguousarray(
        xt.reshape(NG, XG, 128, WPS).transpose(0, 2, 1, 3)
        .reshape(NG, 128, XG * WPS))


def _host_weights(w1, w2):
    # lw1[(j,c),(kw, (r,o))] = w1[o, c, j-r, kw] for j-r in 0..2 else 0
    lw1 = np.zeros((128, 3, 2, 64), np.float32)
    for kw in range(3):
        for r in range(2):
            for j in range(4):
                a = j - r
                if 0 <= a <= 2:
                    # partition j*32+c  ->  col r*64+o
                    lw1[j * 32:(j + 1) * 32, kw, r, :] = w1[:, :, a, kw].T
    lw1 = lw1.reshape(128, 384).astype(np.float16)

    # w2s tiles (f16, unscaled; s1 applied on device)
    w2s = np.zeros((128, 1152), np.float32)
    for kw in range(3):
        A = np.zeros((128, 128), np.float32)
        Bt = np.zeros((128, 128), np.float32)
        for r in range(2):      # input-row half (partition block)
            for u in range(2):  # output-row half (col block)
                # A: input row 2kb+r -> out row 2kb+1+u: kh = r - u
                a = r - u
                if a in (0, 1):
                    A[r * 64:(r + 1) * 64, u * 64:(u + 1) * 64] = \
                        w2[:, :, a, kw].T
                # B: input row 2kb+2+r -> out row 2kb+1+u: kh = r - u + 2
                b_ = r - u + 2
                if 0 <= b_ <= 2:
                    Bt[r * 64:(r + 1) * 64, u * 64:(u + 1) * 64] = \
                        w2[:, :, b_, kw].T
        w2s[:, 128 * kw:128 * kw + 128] = A
        w2s[:, 384 + 128 * kw:384 + 128 * kw + 128] = Bt
        # S0: out row 0, input rows 0,1 (abs) => kh = r+1
        S0 = np.zeros((128, 64), np.float32)
        for r in range(2):
            S0[r * 64:(r + 1) * 64, :] = w2[:, :, r + 1, kw].T
        # S9: out row H-1, input rows H-2,H-1 => kh = r
        S9 = np.zeros((128, 64), np.float32)
        for r in range(2):
            S9[r * 64:(r + 1) * 64, :] = w2[:, :, r, kw].T
        w2s[:, 768 + 64 * kw:768 + 64 * kw + 64] = S0
        w2s[:, 960 + 64 * kw:960 + 64 * kw + 64] = S9
    w2s = w2s.astype(np.float16)
    return lw1, w2s


def _host_aux():
    aux = np.zeros((128, 320), np.float32)
    n_r = NP * W          # 51200 rows-parity count, pass A halves
    aux[np.arange(128), np.arange(128) % 64] = 0.5                  # CMB1
    nB = NPB * W          # 50880
    nS = 2 * W            # 640
    nT = 2 * nB + nS      # 102400
    aux[np.arange(128), 64 + np.arange(128) % 64] = nB / nT         # CMB2
    aux[np.arange(64), 128 + np.arange(64)] = nS / nT               # CMB2s
    aux[np.arange(64)[:, None], 192 + np.arange(128)[None, :]] = (
        (np.arange(128)[None, :] % 64) == np.arange(64)[:, None])   # DUP
    return aux.astype(np.float16)


def _in_map(xi, w1, w2):
    lw1, w2s = _host_weights(w1, w2)
    return {"xg": _tile_x(np.asarray(xi, np.float16)), "lw1": lw1,
            "w2s": w2s, "aux": _host_aux()}


def kernel(x, w1, b1=None, w2=None, b2=None, **kw):
    x = np.ascontiguousarray(np.asarray(x, dtype=np.float32))
    w1 = np.ascontiguousarray(np.asarray(w1, dtype=np.float32))
    w2 = np.ascontiguousarray(np.asarray(w2, dtype=np.float32))
    nc = _get_nc()
    in_maps = [_in_map(x[i], w1, w2) for i in range(B)]
    res = run_bass_kernel_spmd(nc, in_maps, list(range(B)), trace=False)
    return np.stack([res.results[i]["out"].astype(np.float32)
                     for i in range(B)], axis=0)
